# revision 3
# baseline (speedup 1.0000x reference)
"""Trainium2 Bass kernel for nn_DeepKalmanFilter.

Model: bidirectional LSTM over T=256 (B=64, D=128, H=256) followed by a
sampling recurrence over T with S=32 samples, STATE=64.

Strategy (8 NeuronCores, SPMD):
  - Batch-shard everything: core c owns batch rows [8c, 8c+8).
  - All recurrent state kept feature-on-partition ("transposed") so the
    per-step elementwise work runs on 128 partitions.
  - LSTM: both directions advance in the same step loop (independent
    chains); z^T computed as 8 M-chunks x 2 K-chunks of PE matmuls with
    fp8(e4m3) Wh weights (stationary, FWL 4x weight load) against bf16
    hidden state; x@Wx + b precomputed in bulk matmuls into SBUF (bf16).
  - Sampling: state s^T [64, S*B_loc] bf16; p@Wt via K=65 augmented
    matmul (bias row folded in); tanh on ACT; the (tanh + hidden)@W2
    product is computed as two matmuls sharing weights, with hidden
    broadcast over S via a stride-0 access pattern in the rhs; biases
    bmu/bsg folded into the final FMA via scalar_tensor_tensor; eps
    streamed fp32; out written fp32.
  - Host does all pure layout transforms (transposes, gate permutation,
    weight scaling by the 1/3 normalizer, dtype casts, final unshard).
"""

import numpy as np
import ml_dtypes

import concourse.bass as bass
from concourse import bacc, mybir, tile
from concourse.bass_utils import run_bass_kernel_spmd

BF = mybir.dt.bfloat16
F32 = mybir.dt.float32
F8 = mybir.dt.float8e4
AF = mybir.ActivationFunctionType
ALU = mybir.AluOpType

B, D, H, ST = 64, 128, 256, 64
NCORES = 8
BL = B // NCORES          # batch rows per core


def _bcast(ap, reps):
    """Insert a stride-0 dim after the partition dim: [P, ...] -> [P, reps, ...]."""
    return bass.AP(tensor=ap.tensor, offset=ap.offset, ap=[ap.ap[0], [0, reps], *ap.ap[1:]])


def build(T=256, S=32, CH=16, wh_dt=F8):
    """Build + compile the Bass program. Returns nc."""
    SB = S * BL
    nc = bacc.Bacc("TRN2", target_bir_lowering=False, debug=False, num_devices=NCORES)

    y_d = nc.dram_tensor("y", [128, 2, T, BL], BF, kind="ExternalInput")
    eps_d = nc.dram_tensor("eps", [ST, T, SB], F32, kind="ExternalInput")
    wh_d = nc.dram_tensor("wh", [128, 2, 2, 8, 128], wh_dt, kind="ExternalInput")
    wx_d = nc.dram_tensor("wx", [128, 2, 8, 128], BF, kind="ExternalInput")
    b_d = nc.dram_tensor("b", [128, 2, 8], F32, kind="ExternalInput")
    wt_d = nc.dram_tensor("wt", [65, 2, 128], BF, kind="ExternalInput")
    w2_d = nc.dram_tensor("w2", [128, 2, 128], BF, kind="ExternalInput")
    b2_d = nc.dram_tensor("b2", [64, 2], F32, kind="ExternalInput")
    out_d = nc.dram_tensor("out", [ST, T, SB], F32, kind="ExternalOutput")

    with tile.TileContext(nc) as tc:
        with (
            tc.tile_pool(name="const", bufs=1) as const,
            tc.tile_pool(name="lwork", bufs=3) as lwork,
            tc.tile_pool(name="swork", bufs=3) as swork,
            tc.tile_pool(name="epool", bufs=2) as epool,
            tc.tile_pool(name="opool", bufs=2) as opool,
            tc.tile_pool(name="xzp", bufs=2, space="PSUM") as xzp,
            tc.tile_pool(name="zp", bufs=2, space="PSUM") as zp,
            tc.tile_pool(name="ps1p", bufs=2, space="PSUM") as ps1p,
            tc.tile_pool(name="ps2p", bufs=2, space="PSUM") as ps2p,
        ):
            # ---- weight / input loads ----
            wh_sb = const.tile([128, 2, 2, 8, 128], wh_dt)
            nc.sync.dma_start(wh_sb[:], wh_d[:])
            wx_sb = const.tile([128, 2, 8, 128], BF)
            nc.sync.dma_start(wx_sb[:], wx_d[:])
            b_sb = const.tile([128, 2, 8], F32)
            nc.sync.dma_start(b_sb[:], b_d[:])
            wt_sb = const.tile([65, 2, 128], BF)
            nc.sync.dma_start(wt_sb[:], wt_d[:])
            w2_sb = const.tile([128, 2, 128], BF)
            nc.sync.dma_start(w2_sb[:], w2_d[:])
            b2_sb = const.tile([64, 2], F32)
            nc.sync.dma_start(b2_sb[:], b2_d[:])
            y_sb = const.tile([128, 2, T, BL], BF)
            nc.sync.dma_start(y_sb[:], y_d[:])

            # ---- persistent state ----
            xz_sb = const.tile([128, T, 8, 2, BL], BF)
            hidA = const.tile([128, 2, 2, T, BL], BF)      # (ck, dir, t, b)
            zh = const.tile([128, 2, BL], BF)              # zero lstm state
            nc.vector.memset(zh[:], 0.0)
            cA = const.tile([128, 2, 2, BL], F32)          # (ck, dir, b)
            cB = const.tile([128, 2, 2, BL], F32)
            nc.vector.memset(cA[:], 0.0)
            # s-state ring: rows 0:64 = s^T (bf16), row 64 = ones (K=65 bias row)
            sr = [const.tile([65, SB], BF, name=f"sr{i}", tag=f"sr{i}") for i in range(4)]
            for t_ in sr:
                nc.vector.memset(t_[:], 0.0)
                nc.vector.memset(t_[64:65, :], 1.0)

            # ---- phase 0: bulk xz = y @ Wx + b  (per dir; dir 1 is time-reversed y) ----
            NB = (T * BL) // 512 if T * BL >= 512 else 1
            TB = T * BL // NB                # columns (t,b) per block
            TT = TB // BL                    # timesteps per block
            for d in range(2):
                for m in range(8):
                    for nb in range(NB):
                        ps = xzp.tile([128, TT, BL], F32)
                        nc.tensor.matmul(
                            ps[:, :, :],
                            wx_sb[:, d, m, :],
                            y_sb[:, d, nb * TT:(nb + 1) * TT, :],
                            start=True, stop=True,
                        )
                        dst = xz_sb[:, nb * TT:(nb + 1) * TT, m, d, :]
                        bias = b_sb[:, d, m:m + 1]
                        if (m + nb + d) % 2 == 0:
                            nc.scalar.activation(dst, ps[:, :, :], AF.Identity, bias=bias)
                        else:
                            nc.vector.tensor_scalar(dst, ps[:, :, :], bias, None, op0=ALU.add)

            # ---- phase 1: LSTM, both directions per step ----
            for t in range(T):
                z_ps = zp.tile([128, 8, 2, BL], F32)
                for d in range(2):
                    for m in range(8):
                        for kk in range(2):
                            rhs = zh[:, kk, :] if t == 0 else hidA[:, kk, d, t - 1, :]
                            nc.tensor.matmul(
                                z_ps[:, m, d, :],
                                wh_sb[:, d, kk, m, :],
                                rhs,
                                start=(kk == 0), stop=(kk == 1),
                            )
                z_sb = lwork.tile([128, 8, 2, BL], F32)
                nc.vector.tensor_add(z_sb[:], z_ps[:], xz_sb[:, t, :, :, :])
                g_sb = lwork.tile([128, 8, 2, BL], BF)
                nc.scalar.activation(g_sb[:, 0:6, :, :], z_sb[:, 0:6, :, :], AF.Sigmoid)
                nc.scalar.activation(g_sb[:, 6:8, :, :], z_sb[:, 6:8, :, :], AF.Tanh)
                c_prev, c_new = (cA, cB) if t % 2 == 0 else (cB, cA)
                t1g = lwork.tile([128, 2, 2, BL], F32)
                nc.vector.tensor_mul(t1g[:], g_sb[:, 0:2, :, :], g_sb[:, 6:8, :, :])
                t2g = lwork.tile([128, 2, 2, BL], F32)
                nc.vector.tensor_mul(t2g[:], g_sb[:, 2:4, :, :], c_prev[:])
                nc.vector.tensor_add(c_new[:], t1g[:], t2g[:])
                tc_bf = lwork.tile([128, 2, 2, BL], BF)
                nc.scalar.activation(tc_bf[:], c_new[:], AF.Tanh)
                nc.vector.tensor_mul(hidA[:, :, :, t, :], g_sb[:, 4:6, :, :], tc_bf[:])

            # ---- phase 2: sampling recurrence ----
            eps_tl = out_tl = None
            for t in range(T):
                if t % CH == 0:
                    eps_tl = epool.tile([ST, CH, SB], F32)
                    nc.sync.dma_start(eps_tl[:], eps_d[:, t:t + CH, :])
                    out_tl = opool.tile([ST, CH, SB], F32)
                hid_t = swork.tile([128, 2, BL], BF)
                nc.vector.tensor_add(
                    hid_t[:], hidA[:, :, 0, t, :], hidA[:, :, 1, T - 1 - t, :]
                )
                p = sr[3] if t == 0 else sr[(t - 1) % 3]
                ps1 = ps1p.tile([128, 2, SB], F32)
                for hc in range(2):
                    nc.tensor.matmul(
                        ps1[:, hc, :], wt_sb[:, hc, :], p[:, :], start=True, stop=True
                    )
                th = swork.tile([128, 2, SB], BF)
                nc.scalar.activation(th[:], ps1[:], AF.Tanh)
                ps2 = ps2p.tile([128, SB], F32)
                for hc in range(2):
                    nc.tensor.matmul(
                        ps2[:, :], w2_sb[:, hc, :], th[:, hc, :],
                        start=(hc == 0), stop=False,
                    )
                    nc.tensor.matmul(
                        ps2[:, :], w2_sb[:, hc, :], _bcast(hid_t[:, hc, :], S),
                        start=False, stop=(hc == 1),
                    )
                s_cur = sr[t % 3]
                t1 = swork.tile([ST, SB], F32)
                nc.vector.scalar_tensor_tensor(
                    t1[:], ps2[64:128, :], b2_sb[:, 1:2], eps_tl[:, t % CH, :],
                    op0=ALU.add, op1=ALU.mult,
                )
                nc.vector.scalar_tensor_tensor(
                    s_cur[0:64, :], ps2[0:64, :], b2_sb[:, 0:1], t1[:],
                    op0=ALU.add, op1=ALU.add,
                )
                nc.vector.tensor_copy(out_tl[:, t % CH, :], s_cur[0:64, :])
                if t % CH == CH - 1:
                    nc.sync.dma_start(out_d[:, t - CH + 1:t + 1, :], out_tl[:])

    nc.compile()
    return nc


# ------------------------- host-side wrapper -------------------------

_CACHE = {}


def _prep_shared(T, S, Wx_f, Wh_f, b_f, Wx_b, Wh_b, b_b, Wt, bt, Wmu, bmu, Wsg, bsg,
                 wh_np):
    """Weight tensors (identical for every core), laid out SBUF-ready."""
    f32 = np.float32
    # gate permutation [i, f, g, o] -> [i, f, o, g]
    perm = np.r_[0:256, 256:512, 768:1024, 512:768]
    out = {}
    wh = np.empty((128, 2, 2, 8, 128), f32)
    wx = np.empty((128, 2, 8, 128), f32)
    bb = np.empty((128, 2, 8), f32)
    for d, (Wx_, Wh_, b_) in enumerate([(Wx_f, Wh_f, b_f), (Wx_b, Wh_b, b_b)]):
        Wxp, Whp, bp = Wx_[:, perm], Wh_[:, perm], b_[perm]
        for m in range(8):
            wx[:, d, m, :] = Wxp[:, m * 128:(m + 1) * 128]
            bb[:, d, m] = bp[m * 128:(m + 1) * 128]
            for kk in range(2):
                wh[:, d, kk, m, :] = Whp[kk * 128:(kk + 1) * 128, m * 128:(m + 1) * 128]
    out["wh"] = wh.astype(wh_np)
    out["wx"] = wx.astype(ml_dtypes.bfloat16)
    out["b"] = bb
    wt = np.empty((65, 2, 128), f32)
    for hc in range(2):
        wt[0:64, hc, :] = Wt[:, hc * 128:(hc + 1) * 128]
        wt[64, hc, :] = bt[hc * 128:(hc + 1) * 128]
    out["wt"] = wt.astype(ml_dtypes.bfloat16)
    W2 = np.concatenate([Wmu, Wsg], axis=1) / 3.0     # [256, 128]
    w2 = np.empty((128, 2, 128), f32)
    for kk in range(2):
        w2[:, kk, :] = W2[kk * 128:(kk + 1) * 128, :]
    out["w2"] = w2.astype(ml_dtypes.bfloat16)
    out["b2"] = np.stack([bmu, bsg], axis=1).astype(f32)
    return out


def kernel(y, n_samples, eps, Wx_f, Wh_f, b_f, Wx_b, Wh_b, b_b,
           Wt, bt, Wmu, bmu, Wsg, bsg, _trace=False):
    f32 = np.float32
    y = np.asarray(y, f32)
    eps = np.asarray(eps, f32)
    Bn, T, Dn = y.shape
    S = eps.shape[1]
    assert (Bn, Dn) == (B, D)

    key = (T, S)
    if key not in _CACHE:
        _CACHE[key] = build(T=T, S=S)
    nc = _CACHE[key]

    args = [Wx_f, Wh_f, b_f, Wx_b, Wh_b, b_b, Wt, bt, Wmu, bmu, Wsg, bsg]
    args = [np.asarray(a, f32) for a in args]
    shared = _prep_shared(T, S, *args, wh_np=ml_dtypes.float8_e4m3)

    # eps -> [ST, T, S, B] once, then per-core slices
    eps_t = np.ascontiguousarray(eps.transpose(3, 0, 1, 2))   # [64, T, S, B]
    in_maps = []
    for c in range(NCORES):
        bsl = slice(c * BL, (c + 1) * BL)
        y_c = y[bsl].transpose(2, 1, 0)                       # [D, T, BL]
        y_dev = np.empty((128, 2, T, BL), ml_dtypes.bfloat16)
        y_dev[:, 0] = y_c.astype(ml_dtypes.bfloat16)
        y_dev[:, 1] = y_c[:, ::-1].astype(ml_dtypes.bfloat16)
        eps_c = np.ascontiguousarray(eps_t[:, :, :, bsl]).reshape(ST, T, S * BL)
        in_maps.append({"y": y_dev, "eps": eps_c, **shared})

    res = run_bass_kernel_spmd(
        nc, in_maps, core_ids=list(range(NCORES)), trace=_trace
    )
    out = np.empty((S, B, T, ST), f32)
    for c in range(NCORES):
        o = res.results[c]["out"]                             # [ST, T, S*BL]
        out[:, c * BL:(c + 1) * BL] = (
            o.reshape(ST, T, S, BL).transpose(2, 3, 1, 0)
        )
    if _trace:
        kernel._last_results = res
    return out


# revision 5
# speedup vs baseline: 1.2008x; 1.2008x over previous
"""Trainium2 Bass kernel for nn_DeepKalmanFilter.

Model: bidirectional LSTM over T=256 (B=64, D=128, H=256) followed by a
sampling recurrence over T with S=32 samples, STATE=64.

Strategy (8 NeuronCores, SPMD):
  - Batch-shard everything: core c owns batch rows [8c, 8c+8).
  - All recurrent state kept feature-on-partition ("transposed") so the
    per-step elementwise work runs on 128 partitions.
  - LSTM: both directions advance in the same step loop (independent
    chains); z^T computed as 8 M-chunks x 2 K-chunks of PE matmuls with
    fp8(e4m3) Wh weights (stationary, FWL 4x weight load) against bf16
    hidden state; x@Wx + b precomputed in bulk matmuls into SBUF (bf16).
  - Sampling: state s^T [64, S*B_loc] bf16; p@Wt via K=65 augmented
    matmul (bias row folded in); tanh on ACT; the (tanh + hidden)@W2
    product is computed as two matmuls sharing weights, with hidden
    broadcast over S via a stride-0 access pattern in the rhs; biases
    bmu/bsg folded into the final FMA via scalar_tensor_tensor; eps
    streamed fp32; out written fp32.
  - Host does all pure layout transforms (transposes, gate permutation,
    weight scaling by the 1/3 normalizer, dtype casts, final unshard).
"""

import numpy as np
import ml_dtypes

import concourse.bass as bass
from concourse import bacc, mybir, tile
from concourse.bass_utils import run_bass_kernel_spmd

BF = mybir.dt.bfloat16
F32 = mybir.dt.float32
F8 = mybir.dt.float8e4
AF = mybir.ActivationFunctionType
ALU = mybir.AluOpType

B, D, H, ST = 64, 128, 256, 64
NCORES = 8
BL = B // NCORES          # batch rows per core


def _bcast(ap, reps):
    """Insert a stride-0 dim after the partition dim: [P, ...] -> [P, reps, ...]."""
    return bass.AP(tensor=ap.tensor, offset=ap.offset, ap=[ap.ap[0], [0, reps], *ap.ap[1:]])


def build(T=256, S=32, CH=16, wh_dt=F8):
    """Build + compile the Bass program. Returns nc."""
    SB = S * BL
    nc = bacc.Bacc("TRN2", target_bir_lowering=False, debug=False, num_devices=NCORES)

    y_d = nc.dram_tensor("y", [128, 2, T, BL], BF, kind="ExternalInput")
    eps_d = nc.dram_tensor("eps", [ST, T, SB], F32, kind="ExternalInput")
    wh_d = nc.dram_tensor("wh", [128, 2, 2, 8, 128], wh_dt, kind="ExternalInput")
    wx_d = nc.dram_tensor("wx", [128, 2, 8, 128], BF, kind="ExternalInput")
    b_d = nc.dram_tensor("b", [128, 2, 8], F32, kind="ExternalInput")
    wt_d = nc.dram_tensor("wt", [65, 2, 128], BF, kind="ExternalInput")
    w2_d = nc.dram_tensor("w2", [128, 2, 128], BF, kind="ExternalInput")
    b2_d = nc.dram_tensor("b2", [64, 2], F32, kind="ExternalInput")
    out_d = nc.dram_tensor("out", [ST, T, SB], F32, kind="ExternalOutput")

    with tile.TileContext(nc) as tc:
        with (
            tc.tile_pool(name="const", bufs=1) as const,
            tc.tile_pool(name="lwork", bufs=3) as lwork,
            tc.tile_pool(name="swork", bufs=3) as swork,
            tc.tile_pool(name="epool", bufs=2) as epool,
            tc.tile_pool(name="opool", bufs=2) as opool,
        ):
            # ---- weight / input loads ----
            wh_sb = const.tile([128, 2, 2, 8, 128], wh_dt)
            nc.sync.dma_start(wh_sb[:], wh_d[:])
            wx_sb = const.tile([128, 2, 8, 128], BF)
            nc.sync.dma_start(wx_sb[:], wx_d[:])
            b_sb = const.tile([128, 2, 8], F32)
            nc.sync.dma_start(b_sb[:], b_d[:])
            wt_sb = const.tile([65, 2, 128], BF)
            nc.sync.dma_start(wt_sb[:], wt_d[:])
            w2_sb = const.tile([128, 2, 128], BF)
            nc.sync.dma_start(w2_sb[:], w2_d[:])
            b2_sb = const.tile([64, 2], F32)
            nc.sync.dma_start(b2_sb[:], b2_d[:])
            y_sb = const.tile([128, 2, T, BL], BF)
            nc.sync.dma_start(y_sb[:], y_d[:])

            # ---- persistent state ----
            xz_sb = const.tile([128, T, 8, 2, BL], BF)
            hidA = const.tile([128, 2, 2, T, BL], BF)      # (ck, dir, t, b)
            zh = const.tile([128, 2, BL], BF)              # zero lstm state
            nc.vector.memset(zh[:], 0.0)
            cst = [[const.tile([128, 2, BL], F32, name=f"c{d}{i}", tag=f"c{d}{i}")
                    for i in range(2)] for d in range(2)]
            for d in range(2):
                nc.vector.memset(cst[d][0][:], 0.0)
            # s-state rings (per column group): rows 0:64 = s^T, row 64 = ones
            GW = SB // 2                                   # columns per group
            sr = [[const.tile([65, GW], BF, name=f"sr{g}_{i}", tag=f"sr{g}_{i}")
                   for i in range(4)] for g in range(2)]
            for gl in sr:
                for t_ in gl:
                    nc.vector.memset(t_[:], 0.0)
                    nc.vector.memset(t_[64:65, :], 1.0)

            # ---- phase 0: bulk xz = y @ Wx + b  (per dir; dir 1 is time-reversed y)
            # nb outer so early timesteps of both dirs finish first (overlaps LSTM)
            NB = (T * BL) // 512 if T * BL >= 512 else 1
            TB = T * BL // NB                # columns (t,b) per block
            TT = TB // BL                    # timesteps per block
            with tc.tile_pool(name="xzp", bufs=2, space="PSUM") as xzp:
                for nb in range(NB):
                    for d in range(2):
                        for m in range(8):
                            ps = xzp.tile([128, TT, BL], F32)
                            nc.tensor.matmul(
                                ps[:, :, :],
                                wx_sb[:, d, m, :],
                                y_sb[:, d, nb * TT:(nb + 1) * TT, :],
                                start=True, stop=True,
                            )
                            dst = xz_sb[:, nb * TT:(nb + 1) * TT, m, d, :]
                            bias = b_sb[:, d, m:m + 1]
                            if (m + d) % 2 == 0:
                                nc.scalar.activation(dst, ps[:, :, :], AF.Identity, bias=bias)
                            else:
                                nc.vector.tensor_scalar(dst, ps[:, :, :], bias, None, op0=ALU.add)

            # ---- phase 1: LSTM; the two directions are independent chains ----
            with tc.tile_pool(name="zp", bufs=2, space="PSUM") as zp:
                for t in range(T):
                    for d in range(2):
                        z_ps = zp.tile([128, 8, BL], F32, name=f"zps{d}", tag=f"zps{d}")
                        for m in range(8):
                            for kk in range(2):
                                rhs = zh[:, kk, :] if t == 0 else hidA[:, kk, d, t - 1, :]
                                nc.tensor.matmul(
                                    z_ps[:, m, :],
                                    wh_sb[:, d, kk, m, :],
                                    rhs,
                                    start=(kk == 0), stop=(kk == 1),
                                )
                        z_sb = lwork.tile([128, 8, BL], F32, name=f"z{d}", tag=f"z{d}")
                        nc.vector.tensor_add(z_sb[:], z_ps[:], xz_sb[:, t, :, d, :])
                        g_sb = lwork.tile([128, 8, BL], BF, name=f"g{d}", tag=f"g{d}")
                        nc.scalar.activation(g_sb[:, 0:6, :], z_sb[:, 0:6, :], AF.Sigmoid)
                        nc.scalar.activation(g_sb[:, 6:8, :], z_sb[:, 6:8, :], AF.Tanh)
                        c_prev, c_new = cst[d][t % 2], cst[d][(t + 1) % 2]
                        t1g = lwork.tile([128, 2, BL], F32, name=f"t1g{d}", tag=f"t1g{d}")
                        nc.gpsimd.tensor_mul(t1g[:], g_sb[:, 0:2, :], g_sb[:, 6:8, :])
                        t2g = lwork.tile([128, 2, BL], F32, name=f"t2g{d}", tag=f"t2g{d}")
                        nc.vector.tensor_mul(t2g[:], g_sb[:, 2:4, :], c_prev[:])
                        nc.vector.tensor_add(c_new[:], t1g[:], t2g[:])
                        tc_bf = lwork.tile([128, 2, BL], BF, name=f"tc{d}", tag=f"tc{d}")
                        nc.scalar.activation(tc_bf[:], c_new[:], AF.Tanh)
                        nc.vector.tensor_mul(hidA[:, :, d, t, :], g_sb[:, 4:6, :], tc_bf[:])

            # ---- phase 2: sampling; two independent column-group chains ----
            with (
                tc.tile_pool(name="ps1p", bufs=2, space="PSUM") as ps1p,
                tc.tile_pool(name="ps2p", bufs=2, space="PSUM") as ps2p,
            ):
                eps_tl = out_tl = None
                for t in range(T):
                    if t % CH == 0:
                        eps_tl = epool.tile([ST, CH, SB], F32)
                        nc.sync.dma_start(eps_tl[:], eps_d[:, t:t + CH, :])
                        out_tl = opool.tile([ST, CH, SB], F32)
                    hid_t = swork.tile([128, 2, BL], BF)
                    nc.gpsimd.tensor_add(
                        hid_t[:], hidA[:, :, 0, t, :], hidA[:, :, 1, T - 1 - t, :]
                    )
                    for g in range(2):
                        cols = slice(g * GW, (g + 1) * GW)
                        p = sr[g][3] if t == 0 else sr[g][(t - 1) % 3]
                        ps1 = ps1p.tile([128, 2, GW], F32, name=f"ps1{g}", tag=f"ps1{g}")
                        for hc in range(2):
                            nc.tensor.matmul(
                                ps1[:, hc, :], wt_sb[:, hc, :], p[:, :],
                                start=True, stop=True,
                            )
                        th = swork.tile([128, 2, GW], BF, name=f"th{g}", tag=f"th{g}")
                        nc.scalar.activation(th[:], ps1[:], AF.Tanh)
                        ps2 = ps2p.tile([128, GW], F32, name=f"ps2{g}", tag=f"ps2{g}")
                        # hid-broadcast matmuls first (off the s-chain), th last
                        for hc in range(2):
                            nc.tensor.matmul(
                                ps2[:, :], w2_sb[:, hc, :],
                                _bcast(hid_t[:, hc, :], S // 2),
                                start=(hc == 0), stop=False,
                            )
                        for hc in range(2):
                            nc.tensor.matmul(
                                ps2[:, :], w2_sb[:, hc, :], th[:, hc, :],
                                start=False, stop=(hc == 1),
                            )
                        s_cur = sr[g][t % 3]
                        t1 = swork.tile([ST, GW], F32, name=f"t1_{g}", tag=f"t1_{g}")
                        nc.vector.scalar_tensor_tensor(
                            t1[:], ps2[64:128, :], b2_sb[:, 1:2],
                            eps_tl[:, t % CH, cols],
                            op0=ALU.add, op1=ALU.mult,
                        )
                        nc.vector.scalar_tensor_tensor(
                            s_cur[0:64, :], ps2[0:64, :], b2_sb[:, 0:1], t1[:],
                            op0=ALU.add, op1=ALU.add,
                        )
                        nc.gpsimd.tensor_copy(out_tl[:, t % CH, cols], s_cur[0:64, :])
                    if t % CH == CH - 1:
                        nc.sync.dma_start(out_d[:, t - CH + 1:t + 1, :], out_tl[:])

    nc.compile()
    return nc


# ------------------------- host-side wrapper -------------------------

_CACHE = {}


def _prep_shared(T, S, Wx_f, Wh_f, b_f, Wx_b, Wh_b, b_b, Wt, bt, Wmu, bmu, Wsg, bsg,
                 wh_np):
    """Weight tensors (identical for every core), laid out SBUF-ready."""
    f32 = np.float32
    # gate permutation [i, f, g, o] -> [i, f, o, g]
    perm = np.r_[0:256, 256:512, 768:1024, 512:768]
    out = {}
    wh = np.empty((128, 2, 2, 8, 128), f32)
    wx = np.empty((128, 2, 8, 128), f32)
    bb = np.empty((128, 2, 8), f32)
    for d, (Wx_, Wh_, b_) in enumerate([(Wx_f, Wh_f, b_f), (Wx_b, Wh_b, b_b)]):
        Wxp, Whp, bp = Wx_[:, perm], Wh_[:, perm], b_[perm]
        for m in range(8):
            wx[:, d, m, :] = Wxp[:, m * 128:(m + 1) * 128]
            bb[:, d, m] = bp[m * 128:(m + 1) * 128]
            for kk in range(2):
                wh[:, d, kk, m, :] = Whp[kk * 128:(kk + 1) * 128, m * 128:(m + 1) * 128]
    out["wh"] = wh.astype(wh_np)
    out["wx"] = wx.astype(ml_dtypes.bfloat16)
    out["b"] = bb
    wt = np.empty((65, 2, 128), f32)
    for hc in range(2):
        wt[0:64, hc, :] = Wt[:, hc * 128:(hc + 1) * 128]
        wt[64, hc, :] = bt[hc * 128:(hc + 1) * 128]
    out["wt"] = wt.astype(ml_dtypes.bfloat16)
    W2 = np.concatenate([Wmu, Wsg], axis=1) / 3.0     # [256, 128]
    w2 = np.empty((128, 2, 128), f32)
    for kk in range(2):
        w2[:, kk, :] = W2[kk * 128:(kk + 1) * 128, :]
    out["w2"] = w2.astype(ml_dtypes.bfloat16)
    out["b2"] = np.stack([bmu, bsg], axis=1).astype(f32)
    return out


def kernel(y, n_samples, eps, Wx_f, Wh_f, b_f, Wx_b, Wh_b, b_b,
           Wt, bt, Wmu, bmu, Wsg, bsg, _trace=False):
    f32 = np.float32
    y = np.asarray(y, f32)
    eps = np.asarray(eps, f32)
    Bn, T, Dn = y.shape
    S = eps.shape[1]
    assert (Bn, Dn) == (B, D)

    key = (T, S)
    if key not in _CACHE:
        _CACHE[key] = build(T=T, S=S)
    nc = _CACHE[key]

    args = [Wx_f, Wh_f, b_f, Wx_b, Wh_b, b_b, Wt, bt, Wmu, bmu, Wsg, bsg]
    args = [np.asarray(a, f32) for a in args]
    shared = _prep_shared(T, S, *args, wh_np=ml_dtypes.float8_e4m3)

    # eps -> [ST, T, S, B] once, then per-core slices
    eps_t = np.ascontiguousarray(eps.transpose(3, 0, 1, 2))   # [64, T, S, B]
    in_maps = []
    for c in range(NCORES):
        bsl = slice(c * BL, (c + 1) * BL)
        y_c = y[bsl].transpose(2, 1, 0)                       # [D, T, BL]
        y_dev = np.empty((128, 2, T, BL), ml_dtypes.bfloat16)
        y_dev[:, 0] = y_c.astype(ml_dtypes.bfloat16)
        y_dev[:, 1] = y_c[:, ::-1].astype(ml_dtypes.bfloat16)
        eps_c = np.ascontiguousarray(eps_t[:, :, :, bsl]).reshape(ST, T, S * BL)
        in_maps.append({"y": y_dev, "eps": eps_c, **shared})

    res = run_bass_kernel_spmd(
        nc, in_maps, core_ids=list(range(NCORES)), trace=_trace
    )
    out = np.empty((S, B, T, ST), f32)
    for c in range(NCORES):
        o = res.results[c]["out"]                             # [ST, T, S*BL]
        out[:, c * BL:(c + 1) * BL] = (
            o.reshape(ST, T, S, BL).transpose(2, 3, 1, 0)
        )
    if _trace:
        kernel._last_results = res
    return out


# revision 11
# speedup vs baseline: 1.2612x; 1.0503x over previous
"""Trainium2 Bass kernel for nn_DeepKalmanFilter.

Model: bidirectional LSTM over T=256 (B=64, D=128, H=256) followed by a
sampling recurrence over T with S=32 samples, STATE=64.

Strategy (8 NeuronCores, SPMD):
  - Batch-shard everything: core c owns batch rows [8c, 8c+8).
  - All recurrent state kept feature-on-partition ("transposed") so the
    per-step elementwise work runs on 128 partitions.
  - LSTM: both directions advance in the same step loop (independent
    chains); z^T computed as 8 M-chunks x 2 K-chunks of PE matmuls with
    fp8(e4m3) Wh weights (stationary, FWL 4x weight load) against bf16
    hidden state; x@Wx + b precomputed in bulk matmuls into SBUF (bf16).
  - Sampling: state s^T [64, S*B_loc] bf16; p@Wt via K=65 augmented
    matmul (bias row folded in); tanh on ACT; the (tanh + hidden)@W2
    product is computed as two matmuls sharing weights, with hidden
    broadcast over S via a stride-0 access pattern in the rhs; biases
    bmu/bsg folded into the final FMA via scalar_tensor_tensor; eps
    streamed fp32; out written fp32.
  - Host does all pure layout transforms (transposes, gate permutation,
    weight scaling by the 1/3 normalizer, dtype casts, final unshard).
"""

import numpy as np
import ml_dtypes

import concourse.bass as bass
from concourse import bacc, mybir, tile
from concourse.bass_utils import run_bass_kernel_spmd

BF = mybir.dt.bfloat16
F32 = mybir.dt.float32
F8 = mybir.dt.float8e4
AF = mybir.ActivationFunctionType
ALU = mybir.AluOpType

B, D, H, ST = 64, 128, 256, 64
NCORES = 8
BL = B // NCORES          # batch rows per core


def _bcast(ap, reps):
    """Insert a stride-0 dim after the partition dim: [P, ...] -> [P, reps, ...]."""
    return bass.AP(tensor=ap.tensor, offset=ap.offset, ap=[ap.ap[0], [0, reps], *ap.ap[1:]])


def build(T=256, S=32, CH=16, wh_dt=F8):
    """Build + compile the Bass program. Returns nc."""
    SB = S * BL
    nc = bacc.Bacc("TRN2", target_bir_lowering=False, debug=False, num_devices=NCORES)

    y_d = nc.dram_tensor("y", [128, 2, T, BL], BF, kind="ExternalInput")
    eps_d = nc.dram_tensor("eps", [ST, T, SB], F32, kind="ExternalInput")
    wh_d = nc.dram_tensor("wh", [128, 2, 2, 8, 128], wh_dt, kind="ExternalInput")
    wx_d = nc.dram_tensor("wx", [128, 2, 8, 128], BF, kind="ExternalInput")
    b_d = nc.dram_tensor("b", [128, 2, 8], F32, kind="ExternalInput")
    wt_d = nc.dram_tensor("wt", [65, 2, 128], BF, kind="ExternalInput")
    w2_d = nc.dram_tensor("w2", [128, 2, 128], BF, kind="ExternalInput")
    b2_d = nc.dram_tensor("b2", [64, 2], F32, kind="ExternalInput")
    out_d = nc.dram_tensor("out", [ST, T, SB], BF, kind="ExternalOutput")

    with tile.TileContext(nc) as tc:
        with (
            tc.tile_pool(name="const", bufs=1) as const,
            tc.tile_pool(name="lwork", bufs=3) as lwork,
            tc.tile_pool(name="swork", bufs=3) as swork,
            tc.tile_pool(name="epool", bufs=2) as epool,
        ):
            # ---- weight / input loads ----
            wh_sb = const.tile([128, 2, 2, 8, 128], wh_dt)
            nc.sync.dma_start(wh_sb[:], wh_d[:])
            wx_sb = const.tile([128, 2, 8, 128], BF)
            nc.sync.dma_start(wx_sb[:], wx_d[:])
            b_sb = const.tile([128, 2, 8], F32)
            nc.sync.dma_start(b_sb[:], b_d[:])
            wt_sb = const.tile([65, 2, 128], BF)
            nc.sync.dma_start(wt_sb[:], wt_d[:])
            w2_sb = const.tile([128, 2, 128], BF)
            nc.sync.dma_start(w2_sb[:], w2_d[:])
            b2_sb = const.tile([64, 2], F32)
            nc.sync.dma_start(b2_sb[:], b2_d[:])
            y_sb = const.tile([128, 2, T, BL], BF)
            nc.sync.dma_start(y_sb[:], y_d[:])

            # ---- persistent state ----
            xz_sb = const.tile([128, T, 8, 2, BL], BF)
            hidA = const.tile([128, 2, 2, T, BL], BF)      # (ck, dir, t, b)
            zh = const.tile([128, 2, BL], BF)              # zero lstm state
            nc.vector.memset(zh[:], 0.0)
            cst = [[const.tile([128, 2, BL], F32, name=f"c{d}{i}", tag=f"c{d}{i}")
                    for i in range(2)] for d in range(2)]
            for d in range(2):
                nc.vector.memset(cst[d][0][:], 0.0)
            # s-state rings (per column group): rows 0:64 = s^T, row 64 = ones
            GW = SB // 2                                   # columns per group
            sr = [[const.tile([65, GW], BF, name=f"sr{g}_{i}", tag=f"sr{g}_{i}")
                   for i in range(4)] for g in range(2)]
            for gl in sr:
                for t_ in gl:
                    nc.vector.memset(t_[:], 0.0)
                    nc.vector.memset(t_[64:65, :], 1.0)

            # ---- phase 0: bulk xz = y @ Wx + b  (per dir; dir 1 is time-reversed y)
            # nb outer so early timesteps of both dirs finish first (overlaps LSTM)
            NB = (T * BL) // 512 if T * BL >= 512 else 1
            TB = T * BL // NB                # columns (t,b) per block
            TT = TB // BL                    # timesteps per block
            with tc.tile_pool(name="xzp", bufs=2, space="PSUM") as xzp:
                for nb in range(NB):
                    for d in range(2):
                        for m in range(8):
                            ps = xzp.tile([128, TT, BL], F32)
                            nc.tensor.matmul(
                                ps[:, :, :],
                                wx_sb[:, d, m, :],
                                y_sb[:, d, nb * TT:(nb + 1) * TT, :],
                                start=True, stop=True,
                            )
                            dst = xz_sb[:, nb * TT:(nb + 1) * TT, m, d, :]
                            bias = b_sb[:, d, m:m + 1]
                            if (m + d) % 2 == 0:
                                nc.scalar.activation(dst, ps[:, :, :], AF.Identity, bias=bias)
                            else:
                                nc.vector.tensor_scalar(dst, ps[:, :, :], bias, None, op0=ALU.add)

            # ---- phase 1: LSTM; the two directions are independent chains ----
            with tc.tile_pool(name="zp", bufs=2, space="PSUM") as zp:
                for t in range(T):
                    for d in range(2):
                        z_ps = zp.tile([128, 8, BL], F32, name=f"zps{d}", tag=f"zps{d}")
                        for m in range(8):
                            for kk in range(2):
                                rhs = zh[:, kk, :] if t == 0 else hidA[:, kk, d, t - 1, :]
                                nc.tensor.matmul(
                                    z_ps[:, m, :],
                                    wh_sb[:, d, kk, m, :],
                                    rhs,
                                    start=(kk == 0), stop=(kk == 1),
                                )
                        z_sb = lwork.tile([128, 8, BL], F32, name=f"z{d}", tag=f"z{d}")
                        nc.vector.tensor_add(z_sb[:], z_ps[:], xz_sb[:, t, :, d, :])
                        g_sb = lwork.tile([128, 8, BL], BF, name=f"g{d}", tag=f"g{d}")
                        nc.scalar.activation(g_sb[:, 0:6, :], z_sb[:, 0:6, :], AF.Sigmoid)
                        nc.scalar.activation(g_sb[:, 6:8, :], z_sb[:, 6:8, :], AF.Tanh)
                        c_prev, c_new = cst[d][t % 2], cst[d][(t + 1) % 2]
                        t1g = lwork.tile([128, 2, BL], F32, name=f"t1g{d}", tag=f"t1g{d}")
                        nc.vector.tensor_mul(t1g[:], g_sb[:, 0:2, :], g_sb[:, 6:8, :])
                        t2g = lwork.tile([128, 2, BL], F32, name=f"t2g{d}", tag=f"t2g{d}")
                        nc.vector.tensor_mul(t2g[:], g_sb[:, 2:4, :], c_prev[:])
                        nc.vector.tensor_add(c_new[:], t1g[:], t2g[:])
                        tc_bf = lwork.tile([128, 2, BL], BF, name=f"tc{d}", tag=f"tc{d}")
                        nc.scalar.activation(tc_bf[:], c_new[:], AF.Tanh)
                        nc.vector.tensor_mul(hidA[:, :, d, t, :], g_sb[:, 4:6, :], tc_bf[:])

            # ---- phase 2: sampling; two independent column-group chains ----
            with (
                tc.tile_pool(name="ps1p", bufs=2, space="PSUM") as ps1p,
                tc.tile_pool(name="ps2p", bufs=2, space="PSUM") as ps2p,
            ):
                eps_tl = None
                for t in range(T):
                    if t % CH == 0:
                        eps_tl = epool.tile([ST, CH, SB], F32)
                        nc.sync.dma_start(eps_tl[:], eps_d[:, t:t + CH, :])
                    hid_t = swork.tile([128, 2, BL], BF)
                    nc.gpsimd.tensor_add(
                        hid_t[:], hidA[:, :, 0, t, :], hidA[:, :, 1, T - 1 - t, :]
                    )
                    for g in range(2):
                        cols = slice(g * GW, (g + 1) * GW)
                        p = sr[g][3] if t == 0 else sr[g][(t - 1) % 3]
                        ps1 = ps1p.tile([128, 2, GW], F32, name=f"ps1{g}", tag=f"ps1{g}")
                        for hc in range(2):
                            nc.tensor.matmul(
                                ps1[:, hc, :], wt_sb[:, hc, :], p[:, :],
                                start=True, stop=True,
                            )
                        th = swork.tile([128, 2, GW], BF, name=f"th{g}", tag=f"th{g}")
                        nc.scalar.activation(th[:], ps1[:], AF.Tanh)
                        ps2 = ps2p.tile([128, GW], F32, name=f"ps2{g}", tag=f"ps2{g}")
                        # hid-broadcast matmuls first (off the s-chain), th last
                        for hc in range(2):
                            nc.tensor.matmul(
                                ps2[:, :], w2_sb[:, hc, :],
                                _bcast(hid_t[:, hc, :], S // 2),
                                start=(hc == 0), stop=False,
                            )
                        for hc in range(2):
                            nc.tensor.matmul(
                                ps2[:, :], w2_sb[:, hc, :], th[:, hc, :],
                                start=False, stop=(hc == 1),
                            )
                        s_cur = sr[g][t % 3]
                        t1 = swork.tile([ST, GW], F32, name=f"t1_{g}", tag=f"t1_{g}")
                        nc.vector.scalar_tensor_tensor(
                            t1[:], ps2[64:128, :], b2_sb[:, 1:2],
                            eps_tl[:, t % CH, cols],
                            op0=ALU.add, op1=ALU.mult,
                        )
                        nc.vector.scalar_tensor_tensor(
                            s_cur[0:64, :], ps2[0:64, :], b2_sb[:, 0:1], t1[:],
                            op0=ALU.add, op1=ALU.add,
                        )
                        nc.sync.dma_start(out_d[:, t, cols], s_cur[0:64, :])

    nc.compile()
    return nc


# ------------------------- host-side wrapper -------------------------

_CACHE = {}


def _prep_shared(T, S, Wx_f, Wh_f, b_f, Wx_b, Wh_b, b_b, Wt, bt, Wmu, bmu, Wsg, bsg,
                 wh_np):
    """Weight tensors (identical for every core), laid out SBUF-ready."""
    f32 = np.float32
    # gate permutation [i, f, g, o] -> [i, f, o, g]
    perm = np.r_[0:256, 256:512, 768:1024, 512:768]
    out = {}
    wh = np.empty((128, 2, 2, 8, 128), f32)
    wx = np.empty((128, 2, 8, 128), f32)
    bb = np.empty((128, 2, 8), f32)
    for d, (Wx_, Wh_, b_) in enumerate([(Wx_f, Wh_f, b_f), (Wx_b, Wh_b, b_b)]):
        Wxp, Whp, bp = Wx_[:, perm], Wh_[:, perm], b_[perm]
        for m in range(8):
            wx[:, d, m, :] = Wxp[:, m * 128:(m + 1) * 128]
            bb[:, d, m] = bp[m * 128:(m + 1) * 128]
            for kk in range(2):
                wh[:, d, kk, m, :] = Whp[kk * 128:(kk + 1) * 128, m * 128:(m + 1) * 128]
    out["wh"] = wh.astype(wh_np)
    out["wx"] = wx.astype(ml_dtypes.bfloat16)
    out["b"] = bb
    wt = np.empty((65, 2, 128), f32)
    for hc in range(2):
        wt[0:64, hc, :] = Wt[:, hc * 128:(hc + 1) * 128]
        wt[64, hc, :] = bt[hc * 128:(hc + 1) * 128]
    out["wt"] = wt.astype(ml_dtypes.bfloat16)
    W2 = np.concatenate([Wmu, Wsg], axis=1) / 3.0     # [256, 128]
    w2 = np.empty((128, 2, 128), f32)
    for kk in range(2):
        w2[:, kk, :] = W2[kk * 128:(kk + 1) * 128, :]
    out["w2"] = w2.astype(ml_dtypes.bfloat16)
    out["b2"] = np.stack([bmu, bsg], axis=1).astype(f32)
    return out


def kernel(y, n_samples, eps, Wx_f, Wh_f, b_f, Wx_b, Wh_b, b_b,
           Wt, bt, Wmu, bmu, Wsg, bsg, _trace=False):
    f32 = np.float32
    y = np.asarray(y, f32)
    eps = np.asarray(eps, f32)
    Bn, T, Dn = y.shape
    S = eps.shape[1]
    assert (Bn, Dn) == (B, D)

    key = (T, S)
    if key not in _CACHE:
        _CACHE[key] = build(T=T, S=S)
    nc = _CACHE[key]

    args = [Wx_f, Wh_f, b_f, Wx_b, Wh_b, b_b, Wt, bt, Wmu, bmu, Wsg, bsg]
    args = [np.asarray(a, f32) for a in args]
    shared = _prep_shared(T, S, *args, wh_np=ml_dtypes.float8_e4m3)

    # eps -> [ST, T, S, B] once, then per-core slices
    eps_t = np.ascontiguousarray(eps.transpose(3, 0, 1, 2))   # [64, T, S, B]
    in_maps = []
    for c in range(NCORES):
        bsl = slice(c * BL, (c + 1) * BL)
        y_c = y[bsl].transpose(2, 1, 0)                       # [D, T, BL]
        y_dev = np.empty((128, 2, T, BL), ml_dtypes.bfloat16)
        y_dev[:, 0] = y_c.astype(ml_dtypes.bfloat16)
        y_dev[:, 1] = y_c[:, ::-1].astype(ml_dtypes.bfloat16)
        eps_c = np.ascontiguousarray(eps_t[:, :, :, bsl]).reshape(ST, T, S * BL)
        in_maps.append({"y": y_dev, "eps": eps_c, **shared})

    res = run_bass_kernel_spmd(
        nc, in_maps, core_ids=list(range(NCORES)), trace=_trace
    )
    out = np.empty((S, B, T, ST), f32)
    for c in range(NCORES):
        o = np.asarray(res.results[c]["out"], f32)            # [ST, T, S*BL]
        out[:, c * BL:(c + 1) * BL] = (
            o.reshape(ST, T, S, BL).transpose(2, 3, 1, 0)
        )
    if _trace:
        kernel._last_results = res
    return out
